# revision 13
# baseline (speedup 1.0000x reference)
"""Trainium2 Bass kernel for nn_AttentionBlock (ragged_sequence, 16 equal
segments of 2048 q/kv tokens, HID=256, QD=64) on 8 NeuronCores.

Sharding: 2 segments (4096 rows) per core, weights replicated, outputs
concatenated host-side (attention is block-diagonal per segment -> no
cross-core communication needed).

v2: software-pipelined chunks (large pt pool keeps PE dense / HAM warm),
fp8 projections (DoubleRow for QK/Q proj), fp8 P@V, bf16 residual/output,
bf16 epilogue tensors for DVE 4x modes, final scaling on DVE.
"""

import os
import sys

os.environ.setdefault("MYCRO_LOCAL_CACHE", "1")
if "/opt/trn_rl_repo" not in sys.path:
    sys.path.insert(0, "/opt/trn_rl_repo")

import numpy as np

HID = 256
QD = 64
LQ = 2048
LH = 2048
B = 16
NCORES = 8
SEGS = 2                  # segments per core
ROWS = SEGS * LQ          # 4096 q rows per core
EPS = 1e-5
SCALE = 1.0 / 8.0         # 1/sqrt(QD)

_built = {}               # (apply0,) -> nc


def _patch_act_tables():
    """Make the act-table pass choose the combined exp+ln table for every
    activation: blank all other tables (indices preserved so walrus's
    act_func_set_id remap stays correct). Avoids 100+ ACT_TABLE_LOADs
    (1.28us each) from alternating Exp/Ln table picks."""
    import functools
    import concourse.hw_specs as hw_specs
    import concourse.bacc as bacc_mod
    if getattr(hw_specs, "_attn_tables_patched", False):
        return
    orig = hw_specs.get_activation_tables

    @functools.cache
    def patched(arch):
        tabs = dict(orig(arch))
        joint = "natural_log_exp_and_others"
        assert joint in tabs, sorted(tabs)
        return {name: (funcs if name == joint else set())
                for name, funcs in tabs.items()}

    hw_specs.get_activation_tables = patched
    bacc_mod.get_activation_tables = patched
    hw_specs._attn_tables_patched = True


def _build(apply0: bool):
    """Build the per-core Bass graph. apply0: apply norm0 weight/bias on
    device (norm1 weight/bias is applied host-side when non-trivial)."""
    from concourse import bacc, bass, mybir, tile

    _patch_act_tables()

    dt = mybir.dt
    f32 = dt.float32
    bf16 = dt.bfloat16
    f8 = dt.float8e4
    AF = mybir.ActivationFunctionType
    Alu = mybir.AluOpType
    DR = mybir.MatmulPerfMode.DoubleRow

    nc = bacc.Bacc("TRN2", target_bir_lowering=False, debug=False,
                   enable_asserts=False)

    # fp8 pair layouts: [p, ko, x] with contraction index = ko*128 + p
    qTp_d = nc.dram_tensor("qTp", [128, 2, ROWS], f8, kind="ExternalInput")
    hTp_d = nc.dram_tensor("hTp", [128, 2, ROWS], f8, kind="ExternalInput")
    qres_d = nc.dram_tensor("qres", [ROWS, HID], bf16, kind="ExternalInput")
    wqp_d = nc.dram_tensor("WQP", [128, 2, QD], f8, kind="ExternalInput")
    wkp_d = nc.dram_tensor("WKP", [128, 2, QD], f8, kind="ExternalInput")
    wvp_d = nc.dram_tensor("WVP", [128, 2, HID], f8, kind="ExternalInput")
    fwT_d = nc.dram_tensor("FCWT", [HID, HID], bf16, kind="ExternalInput")
    fb_d = nc.dram_tensor("FCB", [1, HID], bf16, kind="ExternalInput")
    idt_d = nc.dram_tensor("IDT", [128, 128], bf16, kind="ExternalInput")
    if apply0:
        n0w_d = nc.dram_tensor("N0W", [128, HID], bf16, kind="ExternalInput")
        n0b_d = nc.dram_tensor("N0B", [128, HID], bf16, kind="ExternalInput")
    out_d = nc.dram_tensor("out", [ROWS, HID], bf16, kind="ExternalOutput")

    qres_a = qres_d.ap()
    out_a = out_d.ap()

    NJT = LH // 128           # 16 j-tiles per segment
    NIC = 2                   # 1024-col i-chunks per segment for scores
    ICW = LQ // NIC           # 1024
    NIL = ICW // 128          # 8 i-tiles per chunk

    with tile.TileContext(nc) as tc:
        with (
            tc.tile_pool(name="const", bufs=1) as cpool,
            tc.tile_pool(name="kqq", bufs=1) as kqq_pool,
            tc.tile_pool(name="vsb", bufs=1) as v_pool,
        ):
            # ---- constants ----
            wqp_sb = cpool.tile([128, 2, QD], f8)
            wkp_sb = cpool.tile([128, 2, QD], f8)
            wvp_sb = cpool.tile([128, 2, HID], f8)
            fw_sb = cpool.tile([128, 2 * HID], bf16)   # fc_w.T chunks
            fb_sb = cpool.tile([1, HID], bf16)
            one_sb = cpool.tile([1, 128], bf16)
            idt_sb = cpool.tile([128, 128], bf16)
            qTp_sb = cpool.tile([128, 2, ROWS], f8)
            hTp_sb = cpool.tile([128, 2, ROWS], f8)
            nc.sync.dma_start(wqp_sb[:], wqp_d.ap()[:, :, :])
            nc.sync.dma_start(wkp_sb[:], wkp_d.ap()[:, :, :])
            nc.sync.dma_start(wvp_sb[:], wvp_d.ap()[:, :, :])
            for e in range(2):
                nc.sync.dma_start(fw_sb[:, e * HID:(e + 1) * HID],
                                  fwT_d.ap()[e * 128:(e + 1) * 128, :])
            nc.sync.dma_start(fb_sb[:], fb_d.ap()[:, :])
            nc.sync.dma_start(idt_sb[:], idt_d.ap()[:, :])
            # split big input loads into column chunks so they spread
            # across DMA queues and the first proj matmuls start early
            for c in range(4):
                cs = slice(c * (ROWS // 4), (c + 1) * (ROWS // 4))
                nc.sync.dma_start(qTp_sb[:, :, cs], qTp_d.ap()[:, :, cs])
                nc.sync.dma_start(hTp_sb[:, :, cs], hTp_d.ap()[:, :, cs])
            nc.vector.memset(one_sb[:], 1.0)
            eps_sb = cpool.tile([128, 1], f32)
            nc.vector.memset(eps_sb[:], EPS)
            nb3_sb = cpool.tile([128, 1], f32)
            nc.vector.memset(nb3_sb[:], -3.0)
            if apply0:
                n0w_sb = cpool.tile([128, HID], bf16)
                n0b_sb = cpool.tile([128, HID], bf16)
                nc.sync.dma_start(n0w_sb[:], n0w_d.ap()[:, :])
                nc.sync.dma_start(n0b_sb[:], n0b_d.ap()[:, :])

            # persistent activations
            kT_sb = kqq_pool.tile([64, ROWS], bf16)     # K^T  [c, j_global]
            qq_sb = kqq_pool.tile([64, ROWS], bf16)     # qq^T [c, i_global]
            # V in jt-pair layout for DoubleRow: [p, pair, parity, d]
            # d 0..255 = V columns, d 256 = ones (softmax denominator),
            # d 257..271 = pad so the parity stride is 16-byte aligned
            VPD = 272
            NPAIR = NJT // 2
            v2_sb = v_pool.tile([128, SEGS * NPAIR, 2, VPD], f8)

            # ---------------- phase 1: projections ----------------
            with (
                tc.tile_pool(name="pp_kq", bufs=4,
                             space=bass.MemorySpace.PSUM) as pp_kq,
                tc.tile_pool(name="pp_v", bufs=4,
                             space=bass.MemorySpace.PSUM) as pp_v,
            ):
                # kT / qq: single DoubleRow matmul per 512-col chunk
                drain_flip = 0
                for dst, w_sb, src in ((kT_sb, wkp_sb, hTp_sb),
                                       (qq_sb, wqp_sb, qTp_sb)):
                    for col in range(0, ROWS, 512):
                        ps = pp_kq.tile([64, 512], f32, tag="kq")
                        nc.tensor.matmul(
                            ps[:], w_sb[:, :, :], src[:, :, col:col + 512],
                            start=True, stop=True, perf_mode=DR)
                        eng = nc.vector if drain_flip % 2 == 0 else nc.scalar
                        if drain_flip % 2 == 0:
                            nc.vector.tensor_copy(dst[:, col:col + 512], ps[:])
                        else:
                            nc.scalar.copy(dst[:, col:col + 512], ps[:])
                        drain_flip += 1

                # V projection (fp8 inputs, accumulate over the two e-halves)
                for s in range(SEGS):
                    for jt in range(NJT):
                        ps = pp_v.tile([128, HID], f32, tag="v")
                        col = s * LH + jt * 128
                        for e in range(2):
                            nc.tensor.matmul(
                                ps[:], hTp_sb[:, e, col:col + 128],
                                wvp_sb[:, e, :],
                                start=(e == 0), stop=(e == 1))
                        dst = v2_sb[:, s * NPAIR + jt // 2, jt % 2, :]
                        if jt % 2 == 0:
                            nc.vector.tensor_copy(dst[:, 0:HID], ps[:])
                        else:
                            nc.scalar.copy(dst[:, 0:HID], ps[:])
                        nc.vector.memset(dst[:, HID:HID + 1], 1.0)

            # ---------------- phase 2: attention + epilogue ----------------
            # d-block split of the [V | ones] columns for the V-stationary
            # DoubleRow P@V matmuls (stationary free dim <= 128)
            DB = [(0, 86), (86, 86), (172, 85)]   # covers 0..256 incl ones
            with (
                tc.tile_pool(name="pt", bufs=18) as pt_pool,
                tc.tile_pool(name="atts", bufs=2) as atts_pool,
                tc.tile_pool(name="qrow", bufs=6) as q_pool,
                tc.tile_pool(name="ep", bufs=6) as ep_pool,
                tc.tile_pool(name="ep8", bufs=12) as ep8_pool,
                tc.tile_pool(name="st8", bufs=10) as st8_pool,
                tc.tile_pool(name="outp", bufs=6) as o_pool,
                tc.tile_pool(name="ps_st", bufs=2,
                             space=bass.MemorySpace.PSUM) as ps_st,
                tc.tile_pool(name="ps_pv", bufs=2,
                             space=bass.MemorySpace.PSUM) as ps_pv,
                tc.tile_pool(name="ps_at", bufs=1,
                             space=bass.MemorySpace.PSUM) as ps_at,
                tc.tile_pool(name="ps_fc", bufs=1,
                             space=bass.MemorySpace.PSUM) as ps_fc,
            ):
                for s in range(SEGS):
                    for ic in range(NIC):
                        icol = s * LQ + ic * ICW
                        # scores^T + exp -> P^T jt-pair tiles (fp8)
                        pt2s = []
                        for jt in range(NJT):
                            st = ps_st.tile([128, ICW], f32, tag="st")
                            for h in range(2):
                                nc.tensor.matmul(
                                    st[:, h * 512:(h + 1) * 512],
                                    kT_sb[:, s * LH + jt * 128:
                                          s * LH + (jt + 1) * 128],
                                    qq_sb[:, icol + h * 512:
                                          icol + (h + 1) * 512],
                                    start=True, stop=True)
                            if jt % 2 == 0:
                                pt2 = pt_pool.tile([128, 2, ICW], f8,
                                                   tag="pt")
                                pt2s.append(pt2)
                            nc.scalar.activation(pt2s[jt // 2][:, jt % 2, :],
                                                 st[:], AF.Exp,
                                                 scale=SCALE, bias=nb3_sb[:])

                        # P@V with V stationary (DoubleRow over jt pairs);
                        # output is att^T [d, i], drained to SBUF via DMA
                        att_sbs = []
                        for db, (dlo, dM) in enumerate(DB):
                            a_sb = atts_pool.tile([128, ICW], bf16,
                                                  tag=f"attT{db}")
                            att_sbs.append(a_sb)
                            pvs = []
                            for _iq in range(2):
                                pv = ps_pv.tile([128, 512], f32, tag="pv")
                                pvs.append(pv)
                            for sp in range(NPAIR):
                                vsl = v2_sb[:, s * NPAIR + sp, :,
                                            dlo:dlo + dM]
                                for iq in range(2):
                                    nc.tensor.matmul(
                                        pvs[iq][0:dM, :], vsl,
                                        pt2s[sp][:, :,
                                                 iq * 512:(iq + 1) * 512],
                                        start=(sp == 0),
                                        stop=(sp == NPAIR - 1),
                                        perf_mode=DR)
                            for iq in range(2):
                                dsl = a_sb[0:dM, iq * 512:(iq + 1) * 512]
                                if (db + iq) % 2 == 0:
                                    nc.vector.tensor_copy(dsl, pvs[iq][0:dM, :])
                                else:
                                    nc.scalar.copy(dsl, pvs[iq][0:dM, :])

                        mva0 = st8_pool.tile([128, 2 * NIL], f32, tag="mva0")
                        xs = []
                        for il in range(NIL):
                            # transpose att^T back to [i, d] (+denominator)
                            att = ps_at.tile([128, HID + 1], bf16, tag="att")
                            for db, (dlo, dM) in enumerate(DB):
                                nc.tensor.transpose(
                                    att[:, dlo:dlo + dM],
                                    att_sbs[db][0:dM,
                                                il * 128:(il + 1) * 128],
                                    idt_sb[0:dM, 0:dM])
                            it = ic * NIL + il
                            row0 = s * LQ + it * 128
                            qt = q_pool.tile([128, HID], bf16, tag="q")
                            nc.sync.dma_start(qt[:], qres_a[row0:row0 + 128, :])
                            # LN is row-scale invariant: x0 = den*q + att
                            # normalizes identically to q + att/den
                            x0 = ep8_pool.tile([128, HID], bf16, tag="x0")
                            nc.vector.scalar_tensor_tensor(
                                x0[:], qt[:], att[:, HID:HID + 1].opt(),
                                att[:, 0:HID],
                                op0=Alu.mult, op1=Alu.add)
                            mv6 = st8_pool.tile([128, 6], f32, tag="mv6")
                            nc.vector.bn_stats(mv6[:], x0[:])
                            nc.vector.bn_aggr(mva0[:, 2 * il:2 * il + 2],
                                              mv6[:])
                            xs.append(x0)

                        ln8a = st8_pool.tile([128, NIL], f32, tag="ln8a")
                        nc.scalar.activation(
                            ln8a[:].rearrange("p (t o) -> p t o", o=1),
                            mva0[:].rearrange("p (t o) -> p t o", o=2)[:, :, 1:2],
                            AF.Ln, bias=eps_sb[:])
                        rstd8a = st8_pool.tile([128, NIL], f32, tag="r8a")
                        nc.scalar.activation(rstd8a[:], ln8a[:], AF.Exp,
                                             scale=-0.5)

                        mva1 = st8_pool.tile([128, 2 * NIL], f32, tag="mva1")
                        ys = []
                        for il in range(NIL):
                            x0 = xs[il]
                            z = ep8_pool.tile([128, HID], bf16, tag="z")
                            nc.vector.tensor_scalar(
                                z[:], x0[:], mva0[:, 2 * il:2 * il + 1].opt(),
                                rstd8a[:, il:il + 1].opt(),
                                op0=Alu.subtract, op1=Alu.mult)
                            if apply0:
                                z2 = ep_pool.tile([128, HID], bf16, tag="z2")
                                nc.gpsimd.tensor_tensor(z2[:], z[:], n0w_sb[:],
                                                        op=Alu.mult)
                                z3 = ep_pool.tile([128, HID], bf16, tag="z3")
                                nc.gpsimd.tensor_tensor(z3[:], z2[:], n0b_sb[:],
                                                        op=Alu.add)
                                zf = z3
                            else:
                                zf = z
                            tp = ps_fc.tile([128, HID], bf16, tag="tpfc")
                            for hh in range(2):
                                nc.tensor.transpose(
                                    tp[:, hh * 128:(hh + 1) * 128],
                                    zf[:, hh * 128:(hh + 1) * 128],
                                    idt_sb[:])
                            zT = ep_pool.tile([128, HID], bf16, tag="zT")
                            nc.vector.tensor_copy(zT[:], tp[:])
                            hres = ps_fc.tile([128, HID], f32, tag="tpfc")
                            nc.tensor.matmul(hres[:], one_sb[:], fb_sb[:],
                                             start=True, stop=False)
                            for hh in range(2):
                                nc.tensor.matmul(
                                    hres[:], zT[:, hh * 128:(hh + 1) * 128],
                                    fw_sb[:, hh * HID:(hh + 1) * HID],
                                    start=False, stop=(hh == 1))
                            y0 = ep8_pool.tile([128, HID], bf16, tag="y0")
                            nc.vector.scalar_tensor_tensor(
                                y0[:], hres[:], 0.0, zf[:],
                                op0=Alu.max, op1=Alu.add)
                            mv6b = st8_pool.tile([128, 6], f32, tag="mv6b")
                            nc.vector.bn_stats(mv6b[:], y0[:])
                            nc.vector.bn_aggr(mva1[:, 2 * il:2 * il + 2],
                                              mv6b[:])
                            ys.append(y0)

                        ln8b = st8_pool.tile([128, NIL], f32, tag="ln8b")
                        nc.scalar.activation(
                            ln8b[:].rearrange("p (t o) -> p t o", o=1),
                            mva1[:].rearrange("p (t o) -> p t o", o=2)[:, :, 1:2],
                            AF.Ln, bias=eps_sb[:])
                        rstd8b = st8_pool.tile([128, NIL], f32, tag="r8b")
                        nc.scalar.activation(rstd8b[:], ln8b[:], AF.Exp,
                                             scale=-0.5)

                        for il in range(NIL):
                            it = ic * NIL + il
                            row0 = s * LQ + it * 128
                            b1 = st8_pool.tile([128, 1], f32, tag="b1")
                            nc.vector.tensor_scalar(
                                b1[:], mva1[:, 2 * il:2 * il + 1],
                                rstd8b[:, il:il + 1].opt(), -1.0,
                                op0=Alu.mult, op1=Alu.mult)
                            ot = o_pool.tile([128, HID], bf16, tag="ot")
                            nc.gpsimd.tensor_scalar(
                                ot[:], ys[il][:],
                                rstd8b[:, il:il + 1].opt(), b1[:].opt(),
                                op0=Alu.mult, op1=Alu.add)
                            nc.sync.dma_start(out_a[row0:row0 + 128, :],
                                              ot[:])

    nc.compile()
    return nc


def _get_nc(apply0: bool):
    key = (bool(apply0),)
    if key not in _built:
        _built[key] = _build(apply0)
    return _built[key]


def _pair(a):
    """[256, X] -> [128, 2, X] with contraction index ko*128 + p."""
    return np.ascontiguousarray(a.reshape(2, 128, -1).transpose(1, 0, 2))


def _shard(inputs, apply0):
    from concourse import mybir
    bf = mybir.dt.np(mybir.dt.bfloat16)
    f8 = mybir.dt.np(mybir.dt.float8e4)

    q = np.ascontiguousarray(np.asarray(inputs["q"], dtype=np.float32))
    h = np.ascontiguousarray(np.asarray(inputs["h"], dtype=np.float32))
    WQ = np.asarray(inputs["WQ"], dtype=np.float32)
    WK = np.asarray(inputs["WK"], dtype=np.float32)
    WV = np.asarray(inputs["WV"], dtype=np.float32)
    fcw = np.asarray(inputs["fc_w"], dtype=np.float32)
    fcb = np.asarray(inputs["fc_b"], dtype=np.float32)

    WQP = _pair(np.ascontiguousarray(WQ.T)).astype(f8)
    WKP = _pair(np.ascontiguousarray(WK.T)).astype(f8)
    WVP = _pair(np.ascontiguousarray(WV.T)).astype(f8)
    FCWT = np.ascontiguousarray(fcw.T).astype(bf)
    FCB = np.ascontiguousarray(fcb.reshape(1, HID)).astype(bf)
    IDT = np.eye(128, dtype=np.float32).astype(bf)

    in_maps = []
    for c in range(NCORES):
        sl = slice(c * ROWS, (c + 1) * ROWS)
        qT = np.ascontiguousarray(q[sl].T)   # [256, ROWS]
        hT = np.ascontiguousarray(h[sl].T)
        m = {
            "qTp": _pair(qT).astype(f8),
            "hTp": _pair(hT).astype(f8),
            "qres": q[sl].astype(bf),
            "WQP": WQP, "WKP": WKP, "WVP": WVP,
            "FCWT": FCWT, "FCB": FCB, "IDT": IDT,
        }
        if apply0:
            m["N0W"] = np.ascontiguousarray(
                np.broadcast_to(np.asarray(inputs["norm0_w"], np.float32),
                                (128, HID))).astype(bf)
            m["N0B"] = np.ascontiguousarray(
                np.broadcast_to(np.asarray(inputs["norm0_b"], np.float32),
                                (128, HID))).astype(bf)
        in_maps.append(m)
    return in_maps


def _run(inputs, trace=False, tmpdir=None):
    from concourse import bass_utils

    n0w = np.asarray(inputs["norm0_w"], np.float32)
    n0b = np.asarray(inputs["norm0_b"], np.float32)
    n1w = np.asarray(inputs["norm1_w"], np.float32)
    n1b = np.asarray(inputs["norm1_b"], np.float32)
    apply0 = not (np.allclose(n0w, 1.0) and np.allclose(n0b, 0.0))
    apply1 = not (np.allclose(n1w, 1.0) and np.allclose(n1b, 0.0))

    nc = _get_nc(apply0)
    in_maps = _shard(inputs, apply0)
    res = bass_utils.run_bass_kernel_spmd(
        nc, in_maps, core_ids=list(range(NCORES)), trace=trace,
        tmpdir=tmpdir)
    out = np.concatenate([np.asarray(res.results[c]["out"])
                          for c in range(NCORES)], axis=0).astype(np.float32)
    if apply1:
        out = out * n1w[None, :] + n1b[None, :]
    return out.astype(np.float32), res


def kernel(**inputs):
    out, _ = _run(inputs, trace=False)
    return out


# revision 14
# speedup vs baseline: 1.3737x; 1.3737x over previous
"""Trainium2 Bass kernel for nn_AttentionBlock (ragged_sequence, 16 equal
segments of 2048 q/kv tokens, HID=256, QD=64) on 8 NeuronCores.

Sharding: 2 segments (4096 rows) per core, weights replicated, outputs
concatenated host-side (attention is block-diagonal per segment -> no
cross-core communication needed).

v2: software-pipelined chunks (large pt pool keeps PE dense / HAM warm),
fp8 projections (DoubleRow for QK/Q proj), fp8 P@V, bf16 residual/output,
bf16 epilogue tensors for DVE 4x modes, final scaling on DVE.
"""

import os
import sys

os.environ.setdefault("MYCRO_LOCAL_CACHE", "1")
if "/opt/trn_rl_repo" not in sys.path:
    sys.path.insert(0, "/opt/trn_rl_repo")

import numpy as np

HID = 256
QD = 64
LQ = 2048
LH = 2048
B = 16
NCORES = 8
SEGS = 2                  # segments per core
ROWS = SEGS * LQ          # 4096 q rows per core
EPS = 1e-5
SCALE = 1.0 / 8.0         # 1/sqrt(QD)

_built = {}               # (apply0,) -> nc


def _patch_act_tables():
    """Make the act-table pass choose the combined exp+ln table for every
    activation: blank all other tables (indices preserved so walrus's
    act_func_set_id remap stays correct). Avoids 100+ ACT_TABLE_LOADs
    (1.28us each) from alternating Exp/Ln table picks."""
    import functools
    import concourse.hw_specs as hw_specs
    import concourse.bacc as bacc_mod
    if getattr(hw_specs, "_attn_tables_patched", False):
        return
    orig = hw_specs.get_activation_tables

    @functools.cache
    def patched(arch):
        tabs = dict(orig(arch))
        joint = "natural_log_exp_and_others"
        assert joint in tabs, sorted(tabs)
        return {name: (funcs if name == joint else set())
                for name, funcs in tabs.items()}

    hw_specs.get_activation_tables = patched
    bacc_mod.get_activation_tables = patched
    hw_specs._attn_tables_patched = True


def _build(apply0: bool):
    """Build the per-core Bass graph. apply0: apply norm0 weight/bias on
    device (norm1 weight/bias is applied host-side when non-trivial)."""
    from concourse import bacc, bass, mybir, tile

    _patch_act_tables()

    dt = mybir.dt
    f32 = dt.float32
    bf16 = dt.bfloat16
    f8 = dt.float8e4
    AF = mybir.ActivationFunctionType
    Alu = mybir.AluOpType
    DR = mybir.MatmulPerfMode.DoubleRow

    nc = bacc.Bacc("TRN2", target_bir_lowering=False, debug=False,
                   enable_asserts=False)

    # fp8 pair layouts: [p, ko, x] with contraction index = ko*128 + p
    qTp_d = nc.dram_tensor("qTp", [128, 2, ROWS], f8, kind="ExternalInput")
    hTp_d = nc.dram_tensor("hTp", [128, 2, ROWS], f8, kind="ExternalInput")
    qres_d = nc.dram_tensor("qres", [ROWS, HID], bf16, kind="ExternalInput")
    wqp_d = nc.dram_tensor("WQP", [128, 2, QD], f8, kind="ExternalInput")
    wkp_d = nc.dram_tensor("WKP", [128, 2, QD], f8, kind="ExternalInput")
    wvp_d = nc.dram_tensor("WVP", [128, 2, HID], f8, kind="ExternalInput")
    fwT_d = nc.dram_tensor("FCWT", [HID, HID], bf16, kind="ExternalInput")
    fb_d = nc.dram_tensor("FCB", [1, HID], bf16, kind="ExternalInput")
    idt_d = nc.dram_tensor("IDT", [128, 128], bf16, kind="ExternalInput")
    if apply0:
        n0w_d = nc.dram_tensor("N0W", [128, HID], bf16, kind="ExternalInput")
        n0b_d = nc.dram_tensor("N0B", [128, HID], bf16, kind="ExternalInput")
    out_d = nc.dram_tensor("out", [ROWS, HID], bf16, kind="ExternalOutput")

    qres_a = qres_d.ap()
    out_a = out_d.ap()

    NJT = LH // 128           # 16 j-tiles per segment
    NIC = 2                   # 1024-col i-chunks per segment for scores
    ICW = LQ // NIC           # 1024
    NIL = ICW // 128          # 8 i-tiles per chunk

    with tile.TileContext(nc) as tc:
        with (
            tc.tile_pool(name="const", bufs=1) as cpool,
            tc.tile_pool(name="kqq", bufs=1) as kqq_pool,
            tc.tile_pool(name="vsb", bufs=1) as v_pool,
        ):
            # ---- constants ----
            wqp_sb = cpool.tile([128, 2, QD], f8)
            wkp_sb = cpool.tile([128, 2, QD], f8)
            wvp_sb = cpool.tile([128, 2, HID], f8)
            fw_sb = cpool.tile([128, 2 * HID], bf16)   # fc_w.T chunks
            fb_sb = cpool.tile([1, HID], bf16)
            one_sb = cpool.tile([1, 128], bf16)
            idt_sb = cpool.tile([128, 128], bf16)
            qTp_sb = cpool.tile([128, 2, ROWS], f8)
            hTp_sb = cpool.tile([128, 2, ROWS], f8)
            nc.sync.dma_start(wqp_sb[:], wqp_d.ap()[:, :, :])
            nc.sync.dma_start(wkp_sb[:], wkp_d.ap()[:, :, :])
            nc.sync.dma_start(wvp_sb[:], wvp_d.ap()[:, :, :])
            for e in range(2):
                nc.sync.dma_start(fw_sb[:, e * HID:(e + 1) * HID],
                                  fwT_d.ap()[e * 128:(e + 1) * 128, :])
            nc.sync.dma_start(fb_sb[:], fb_d.ap()[:, :])
            nc.sync.dma_start(idt_sb[:], idt_d.ap()[:, :])
            # split big input loads into column chunks so they spread
            # across DMA queues and the first proj matmuls start early
            for c in range(4):
                cs = slice(c * (ROWS // 4), (c + 1) * (ROWS // 4))
                nc.sync.dma_start(qTp_sb[:, :, cs], qTp_d.ap()[:, :, cs])
                nc.sync.dma_start(hTp_sb[:, :, cs], hTp_d.ap()[:, :, cs])
            nc.vector.memset(one_sb[:], 1.0)
            eps_sb = cpool.tile([128, 1], f32)
            nc.vector.memset(eps_sb[:], EPS)
            nb3_sb = cpool.tile([128, 1], f32)
            nc.vector.memset(nb3_sb[:], -3.0)
            if apply0:
                n0w_sb = cpool.tile([128, HID], bf16)
                n0b_sb = cpool.tile([128, HID], bf16)
                nc.sync.dma_start(n0w_sb[:], n0w_d.ap()[:, :])
                nc.sync.dma_start(n0b_sb[:], n0b_d.ap()[:, :])

            # persistent activations
            kT_sb = kqq_pool.tile([64, ROWS], bf16)     # K^T  [c, j_global]
            qq_sb = kqq_pool.tile([64, ROWS], bf16)     # qq^T [c, i_global]
            # V in jt-pair layout for DoubleRow: [p, pair, parity, d]
            # d 0..255 = V columns, d 256 = ones (softmax denominator),
            # d 257..271 = pad so the parity stride is 16-byte aligned
            VPD = 272
            NPAIR = NJT // 2
            v2_sb = v_pool.tile([128, SEGS * NPAIR, 2, VPD], f8)

            # ---------------- phase 1: projections ----------------
            with (
                tc.tile_pool(name="pp_kq", bufs=4,
                             space=bass.MemorySpace.PSUM) as pp_kq,
                tc.tile_pool(name="pp_v", bufs=4,
                             space=bass.MemorySpace.PSUM) as pp_v,
            ):
                # kT / qq: single DoubleRow matmul per 512-col chunk
                drain_flip = 0
                for dst, w_sb, src in ((kT_sb, wkp_sb, hTp_sb),
                                       (qq_sb, wqp_sb, qTp_sb)):
                    for col in range(0, ROWS, 512):
                        ps = pp_kq.tile([64, 512], f32, tag="kq")
                        nc.tensor.matmul(
                            ps[:], w_sb[:, :, :], src[:, :, col:col + 512],
                            start=True, stop=True, perf_mode=DR)
                        eng = nc.vector if drain_flip % 2 == 0 else nc.scalar
                        if drain_flip % 2 == 0:
                            nc.vector.tensor_copy(dst[:, col:col + 512], ps[:])
                        else:
                            nc.scalar.copy(dst[:, col:col + 512], ps[:])
                        drain_flip += 1

                # V projection (fp8 inputs, accumulate over the two e-halves)
                for s in range(SEGS):
                    for jt in range(NJT):
                        ps = pp_v.tile([128, HID], f32, tag="v")
                        col = s * LH + jt * 128
                        for e in range(2):
                            nc.tensor.matmul(
                                ps[:], hTp_sb[:, e, col:col + 128],
                                wvp_sb[:, e, :],
                                start=(e == 0), stop=(e == 1))
                        dst = v2_sb[:, s * NPAIR + jt // 2, jt % 2, :]
                        if jt % 2 == 0:
                            nc.vector.tensor_copy(dst[:, 0:HID], ps[:])
                        else:
                            nc.scalar.copy(dst[:, 0:HID], ps[:])
                        nc.vector.memset(dst[:, HID:HID + 1], 1.0)

            # ---------------- phase 2: attention + epilogue ----------------
            with (
                tc.tile_pool(name="pt", bufs=18) as pt_pool,
                tc.tile_pool(name="qrow", bufs=6) as q_pool,
                tc.tile_pool(name="ep", bufs=10) as ep_pool,
                tc.tile_pool(name="ep8", bufs=12) as ep8_pool,
                tc.tile_pool(name="st8", bufs=10) as st8_pool,
                tc.tile_pool(name="outp", bufs=6) as o_pool,
                tc.tile_pool(name="ps_st", bufs=2,
                             space=bass.MemorySpace.PSUM) as ps_st,
                tc.tile_pool(name="ps_att", bufs=2,
                             space=bass.MemorySpace.PSUM) as ps_att,
                tc.tile_pool(name="ps_fc", bufs=1,
                             space=bass.MemorySpace.PSUM) as ps_fc,
                tc.tile_pool(name="ps_tp", bufs=1,
                             space=bass.MemorySpace.PSUM) as ps_tp,
            ):
                chunks = [(s, ic) for s in range(SEGS) for ic in range(NIC)]
                pts = {}     # ci -> list of NPAIR paired P^T tiles

                def emit_score_exp(ci, jt):
                    s, ic = chunks[ci]
                    icol = s * LQ + ic * ICW
                    st = ps_st.tile([128, ICW], f32, tag="st")
                    for h in range(2):
                        nc.tensor.matmul(
                            st[:, h * 512:(h + 1) * 512],
                            kT_sb[:, s * LH + jt * 128:
                                  s * LH + (jt + 1) * 128],
                            qq_sb[:, icol + h * 512:icol + (h + 1) * 512],
                            start=True, stop=True)
                    if jt % 2 == 0:
                        pt2 = pt_pool.tile([128, 2, ICW], f8, tag="pt")
                        pts[ci].append(pt2)
                    nc.scalar.activation(pts[ci][jt // 2][:, jt % 2, :],
                                         st[:], AF.Exp,
                                         scale=SCALE, bias=nb3_sb[:])

                # prologue: first chunk's scores+exp
                pts[0] = []
                for jt in range(NJT):
                    emit_score_exp(0, jt)

                for ci, (s, ic) in enumerate(chunks):
                    if ci + 1 < len(chunks):
                        pts[ci + 1] = []
                    mva0 = st8_pool.tile([128, 2 * NIL], f32, tag="mva0")
                    xs = []
                    for il in range(NIL):
                        # P@V: DoubleRow over jt pairs, P^T pair stationary
                        att = ps_att.tile([128, HID + 1], f32, tag="att")
                        for sp in range(NPAIR):
                            nc.tensor.matmul(
                                att[:],
                                pts[ci][sp][:, :, il * 128:(il + 1) * 128],
                                v2_sb[:, s * NPAIR + sp, :, 0:HID + 1],
                                start=(sp == 0), stop=(sp == NPAIR - 1),
                                perf_mode=DR)
                        it = ic * NIL + il
                        row0 = s * LQ + it * 128
                        qt = q_pool.tile([128, HID], bf16, tag="q")
                        nc.sync.dma_start(qt[:], qres_a[row0:row0 + 128, :])
                        # LN is row-scale invariant: x0 = den*q + att
                        # normalizes identically to q + att/den
                        x0 = ep8_pool.tile([128, HID], bf16, tag="x0")
                        nc.vector.scalar_tensor_tensor(
                            x0[:], qt[:], att[:, HID:HID + 1].opt(),
                            att[:, 0:HID],
                            op0=Alu.mult, op1=Alu.add)
                        mv6 = st8_pool.tile([128, 6], f32, tag="mv6")
                        nc.vector.bn_stats(mv6[:], x0[:])
                        nc.vector.bn_aggr(mva0[:, 2 * il:2 * il + 2],
                                          mv6[:])
                        xs.append(x0)
                        # interleave next chunk's scores+exp so the PE and
                        # ACT streams stay dense across the chunk boundary
                        if ci + 1 < len(chunks):
                            emit_score_exp(ci + 1, 2 * il)
                            emit_score_exp(ci + 1, 2 * il + 1)

                    ln8a = st8_pool.tile([128, NIL], f32, tag="ln8a")
                    nc.scalar.activation(
                        ln8a[:].rearrange("p (t o) -> p t o", o=1),
                        mva0[:].rearrange("p (t o) -> p t o", o=2)[:, :, 1:2],
                        AF.Ln, bias=eps_sb[:])
                    rstd8a = st8_pool.tile([128, NIL], f32, tag="r8a")
                    nc.scalar.activation(rstd8a[:], ln8a[:], AF.Exp,
                                         scale=-0.5)

                    mva1 = st8_pool.tile([128, 2 * NIL], f32, tag="mva1")
                    ys = []
                    for il in range(NIL):
                        x0 = xs[il]
                        z = ep8_pool.tile([128, HID], bf16, tag="z")
                        nc.vector.tensor_scalar(
                            z[:], x0[:], mva0[:, 2 * il:2 * il + 1].opt(),
                            rstd8a[:, il:il + 1].opt(),
                            op0=Alu.subtract, op1=Alu.mult)
                        if apply0:
                            z2 = ep_pool.tile([128, HID], bf16, tag="z2")
                            nc.gpsimd.tensor_tensor(z2[:], z[:], n0w_sb[:],
                                                    op=Alu.mult)
                            z3 = ep_pool.tile([128, HID], bf16, tag="z3")
                            nc.gpsimd.tensor_tensor(z3[:], z2[:], n0b_sb[:],
                                                    op=Alu.add)
                            zf = z3
                        else:
                            zf = z
                        hres = ps_fc.tile([128, HID], f32, tag="fc")
                        nc.tensor.matmul(hres[:], one_sb[:], fb_sb[:],
                                         start=True, stop=False)
                        tp = ps_tp.tile([128, HID], bf16, tag="tp")
                        for hh in range(2):
                            nc.tensor.transpose(
                                tp[:, hh * 128:(hh + 1) * 128],
                                zf[:, hh * 128:(hh + 1) * 128],
                                idt_sb[:])
                        zT = ep_pool.tile([128, HID], bf16, tag="zT")
                        nc.vector.tensor_copy(zT[:], tp[:])
                        for hh in range(2):
                            nc.tensor.matmul(
                                hres[:], zT[:, hh * 128:(hh + 1) * 128],
                                fw_sb[:, hh * HID:(hh + 1) * HID],
                                start=False, stop=(hh == 1))
                        y0 = ep8_pool.tile([128, HID], bf16, tag="y0")
                        nc.vector.scalar_tensor_tensor(
                            y0[:], hres[:], 0.0, zf[:],
                            op0=Alu.max, op1=Alu.add)
                        mv6b = st8_pool.tile([128, 6], f32, tag="mv6b")
                        nc.vector.bn_stats(mv6b[:], y0[:])
                        nc.vector.bn_aggr(mva1[:, 2 * il:2 * il + 2],
                                          mv6b[:])
                        ys.append(y0)

                    ln8b = st8_pool.tile([128, NIL], f32, tag="ln8b")
                    nc.scalar.activation(
                        ln8b[:].rearrange("p (t o) -> p t o", o=1),
                        mva1[:].rearrange("p (t o) -> p t o", o=2)[:, :, 1:2],
                        AF.Ln, bias=eps_sb[:])
                    rstd8b = st8_pool.tile([128, NIL], f32, tag="r8b")
                    nc.scalar.activation(rstd8b[:], ln8b[:], AF.Exp,
                                         scale=-0.5)

                    for il in range(NIL):
                        it = ic * NIL + il
                        row0 = s * LQ + it * 128
                        b1 = st8_pool.tile([128, 1], f32, tag="b1")
                        nc.vector.tensor_scalar(
                            b1[:], mva1[:, 2 * il:2 * il + 1],
                            rstd8b[:, il:il + 1].opt(), -1.0,
                            op0=Alu.mult, op1=Alu.mult)
                        ot = o_pool.tile([128, HID], bf16, tag="ot")
                        nc.vector.tensor_scalar(
                            ot[:], ys[il][:],
                            rstd8b[:, il:il + 1].opt(), b1[:].opt(),
                            op0=Alu.mult, op1=Alu.add)
                        nc.sync.dma_start(out_a[row0:row0 + 128, :],
                                          ot[:])
                    del pts[ci]

    nc.compile()
    return nc


def _get_nc(apply0: bool):
    key = (bool(apply0),)
    if key not in _built:
        _built[key] = _build(apply0)
    return _built[key]


def _pair(a):
    """[256, X] -> [128, 2, X] with contraction index ko*128 + p."""
    return np.ascontiguousarray(a.reshape(2, 128, -1).transpose(1, 0, 2))


def _shard(inputs, apply0):
    from concourse import mybir
    bf = mybir.dt.np(mybir.dt.bfloat16)
    f8 = mybir.dt.np(mybir.dt.float8e4)

    q = np.ascontiguousarray(np.asarray(inputs["q"], dtype=np.float32))
    h = np.ascontiguousarray(np.asarray(inputs["h"], dtype=np.float32))
    WQ = np.asarray(inputs["WQ"], dtype=np.float32)
    WK = np.asarray(inputs["WK"], dtype=np.float32)
    WV = np.asarray(inputs["WV"], dtype=np.float32)
    fcw = np.asarray(inputs["fc_w"], dtype=np.float32)
    fcb = np.asarray(inputs["fc_b"], dtype=np.float32)

    WQP = _pair(np.ascontiguousarray(WQ.T)).astype(f8)
    WKP = _pair(np.ascontiguousarray(WK.T)).astype(f8)
    WVP = _pair(np.ascontiguousarray(WV.T)).astype(f8)
    FCWT = np.ascontiguousarray(fcw.T).astype(bf)
    FCB = np.ascontiguousarray(fcb.reshape(1, HID)).astype(bf)
    IDT = np.eye(128, dtype=np.float32).astype(bf)

    in_maps = []
    for c in range(NCORES):
        sl = slice(c * ROWS, (c + 1) * ROWS)
        qT = np.ascontiguousarray(q[sl].T)   # [256, ROWS]
        hT = np.ascontiguousarray(h[sl].T)
        m = {
            "qTp": _pair(qT).astype(f8),
            "hTp": _pair(hT).astype(f8),
            "qres": q[sl].astype(bf),
            "WQP": WQP, "WKP": WKP, "WVP": WVP,
            "FCWT": FCWT, "FCB": FCB, "IDT": IDT,
        }
        if apply0:
            m["N0W"] = np.ascontiguousarray(
                np.broadcast_to(np.asarray(inputs["norm0_w"], np.float32),
                                (128, HID))).astype(bf)
            m["N0B"] = np.ascontiguousarray(
                np.broadcast_to(np.asarray(inputs["norm0_b"], np.float32),
                                (128, HID))).astype(bf)
        in_maps.append(m)
    return in_maps


def _run(inputs, trace=False, tmpdir=None):
    from concourse import bass_utils

    n0w = np.asarray(inputs["norm0_w"], np.float32)
    n0b = np.asarray(inputs["norm0_b"], np.float32)
    n1w = np.asarray(inputs["norm1_w"], np.float32)
    n1b = np.asarray(inputs["norm1_b"], np.float32)
    apply0 = not (np.allclose(n0w, 1.0) and np.allclose(n0b, 0.0))
    apply1 = not (np.allclose(n1w, 1.0) and np.allclose(n1b, 0.0))

    nc = _get_nc(apply0)
    in_maps = _shard(inputs, apply0)
    res = bass_utils.run_bass_kernel_spmd(
        nc, in_maps, core_ids=list(range(NCORES)), trace=trace,
        tmpdir=tmpdir)
    out = np.concatenate([np.asarray(res.results[c]["out"])
                          for c in range(NCORES)], axis=0).astype(np.float32)
    if apply1:
        out = out * n1w[None, :] + n1b[None, :]
    return out.astype(np.float32), res


def kernel(**inputs):
    out, _ = _run(inputs, trace=False)
    return out


# revision 16
# speedup vs baseline: 1.4227x; 1.0356x over previous
"""Trainium2 Bass kernel for nn_AttentionBlock (ragged_sequence, 16 equal
segments of 2048 q/kv tokens, HID=256, QD=64) on 8 NeuronCores.

Sharding: 2 segments (4096 rows) per core, weights replicated, outputs
concatenated host-side (attention is block-diagonal per segment -> no
cross-core communication needed).

v2: software-pipelined chunks (large pt pool keeps PE dense / HAM warm),
fp8 projections (DoubleRow for QK/Q proj), fp8 P@V, bf16 residual/output,
bf16 epilogue tensors for DVE 4x modes, final scaling on DVE.
"""

import os
import sys

os.environ.setdefault("MYCRO_LOCAL_CACHE", "1")
if "/opt/trn_rl_repo" not in sys.path:
    sys.path.insert(0, "/opt/trn_rl_repo")

import numpy as np

HID = 256
QD = 64
LQ = 2048
LH = 2048
B = 16
NCORES = 8
SEGS = 2                  # segments per core
ROWS = SEGS * LQ          # 4096 q rows per core
EPS = 1e-5
SCALE = 1.0 / 8.0         # 1/sqrt(QD)

_built = {}               # (apply0,) -> nc


def _patch_act_tables():
    """Make the act-table pass choose the combined exp+ln table for every
    activation: blank all other tables (indices preserved so walrus's
    act_func_set_id remap stays correct). Avoids 100+ ACT_TABLE_LOADs
    (1.28us each) from alternating Exp/Ln table picks."""
    import functools
    import concourse.hw_specs as hw_specs
    import concourse.bacc as bacc_mod
    if getattr(hw_specs, "_attn_tables_patched", False):
        return
    orig = hw_specs.get_activation_tables

    @functools.cache
    def patched(arch):
        tabs = dict(orig(arch))
        joint = "natural_log_exp_and_others"
        assert joint in tabs, sorted(tabs)
        return {name: (funcs if name == joint else set())
                for name, funcs in tabs.items()}

    hw_specs.get_activation_tables = patched
    bacc_mod.get_activation_tables = patched
    hw_specs._attn_tables_patched = True


def _build(apply0: bool):
    """Build the per-core Bass graph. apply0: apply norm0 weight/bias on
    device (norm1 weight/bias is applied host-side when non-trivial)."""
    from concourse import bacc, bass, mybir, tile

    _patch_act_tables()

    dt = mybir.dt
    f32 = dt.float32
    bf16 = dt.bfloat16
    f8 = dt.float8e4
    AF = mybir.ActivationFunctionType
    Alu = mybir.AluOpType
    DR = mybir.MatmulPerfMode.DoubleRow

    nc = bacc.Bacc("TRN2", target_bir_lowering=False, debug=False,
                   enable_asserts=False)

    # fp8 pair layouts: [p, ko, x] with contraction index = ko*128 + p
    qTp_d = nc.dram_tensor("qTp", [128, 2, ROWS], f8, kind="ExternalInput")
    hTp_d = nc.dram_tensor("hTp", [128, 2, ROWS], f8, kind="ExternalInput")
    qres_d = nc.dram_tensor("qres", [ROWS, HID], bf16, kind="ExternalInput")
    wqp_d = nc.dram_tensor("WQP", [128, 2, QD], f8, kind="ExternalInput")
    wkp_d = nc.dram_tensor("WKP", [128, 2, QD], f8, kind="ExternalInput")
    wvp_d = nc.dram_tensor("WVP", [128, 2, HID], f8, kind="ExternalInput")
    fwT_d = nc.dram_tensor("FCWT", [HID, HID], bf16, kind="ExternalInput")
    fb_d = nc.dram_tensor("FCB", [1, HID], bf16, kind="ExternalInput")
    idt_d = nc.dram_tensor("IDT", [128, 128], bf16, kind="ExternalInput")
    if apply0:
        n0w_d = nc.dram_tensor("N0W", [128, HID], bf16, kind="ExternalInput")
        n0b_d = nc.dram_tensor("N0B", [128, HID], bf16, kind="ExternalInput")
    out_d = nc.dram_tensor("out", [ROWS, HID], bf16, kind="ExternalOutput")

    qres_a = qres_d.ap()
    out_a = out_d.ap()

    NJT = LH // 128           # 16 j-tiles per segment
    NIC = 2                   # 1024-col i-chunks per segment for scores
    ICW = LQ // NIC           # 1024
    NIL = ICW // 128          # 8 i-tiles per chunk

    with tile.TileContext(nc) as tc:
        with (
            tc.tile_pool(name="const", bufs=1) as cpool,
            tc.tile_pool(name="kqq", bufs=1) as kqq_pool,
            tc.tile_pool(name="vsb", bufs=1) as v_pool,
        ):
            # ---- constants ----
            wqp_sb = cpool.tile([128, 2, QD], f8)
            wkp_sb = cpool.tile([128, 2, QD], f8)
            wvp_sb = cpool.tile([128, 2, HID], f8)
            fw_sb = cpool.tile([128, 2 * HID], bf16)   # fc_w.T chunks
            fb_sb = cpool.tile([1, HID], bf16)
            one_sb = cpool.tile([1, 128], bf16)
            idt_sb = cpool.tile([128, 128], bf16)
            qTp_sb = cpool.tile([128, 2, ROWS], f8)
            hTp_sb = cpool.tile([128, 2, ROWS], f8)
            nc.sync.dma_start(wqp_sb[:], wqp_d.ap()[:, :, :])
            nc.sync.dma_start(wkp_sb[:], wkp_d.ap()[:, :, :])
            nc.sync.dma_start(wvp_sb[:], wvp_d.ap()[:, :, :])
            for e in range(2):
                nc.sync.dma_start(fw_sb[:, e * HID:(e + 1) * HID],
                                  fwT_d.ap()[e * 128:(e + 1) * 128, :])
            nc.sync.dma_start(fb_sb[:], fb_d.ap()[:, :])
            nc.sync.dma_start(idt_sb[:], idt_d.ap()[:, :])
            # split big input loads into column chunks so they spread
            # across DMA queues and the first proj matmuls start early
            for c in range(4):
                cs = slice(c * (ROWS // 4), (c + 1) * (ROWS // 4))
                nc.sync.dma_start(qTp_sb[:, :, cs], qTp_d.ap()[:, :, cs])
                nc.sync.dma_start(hTp_sb[:, :, cs], hTp_d.ap()[:, :, cs])
            nc.vector.memset(one_sb[:], 1.0)
            eps_sb = cpool.tile([128, 1], f32)
            nc.vector.memset(eps_sb[:], EPS)
            nb3_sb = cpool.tile([128, 1], f32)
            nc.vector.memset(nb3_sb[:], -3.0)
            if apply0:
                n0w_sb = cpool.tile([128, HID], bf16)
                n0b_sb = cpool.tile([128, HID], bf16)
                nc.sync.dma_start(n0w_sb[:], n0w_d.ap()[:, :])
                nc.sync.dma_start(n0b_sb[:], n0b_d.ap()[:, :])

            # persistent activations
            kT_sb = kqq_pool.tile([64, ROWS], bf16)     # K^T  [c, j_global]
            qq_sb = kqq_pool.tile([64, ROWS], bf16)     # qq^T [c, i_global]
            # V in jt-pair layout for DoubleRow: [p, pair, parity, d]
            # d 0..255 = V columns, d 256 = ones (softmax denominator),
            # d 257..271 = pad so the parity stride is 16-byte aligned
            VPD = 272
            NPAIR = NJT // 2
            v2_sb = v_pool.tile([128, SEGS * NPAIR, 2, VPD], f8)

            # ---------------- phase 1: projections ----------------
            with (
                tc.tile_pool(name="pp_kq", bufs=2,
                             space=bass.MemorySpace.PSUM) as pp_kq,
                tc.tile_pool(name="pp_v", bufs=4,
                             space=bass.MemorySpace.PSUM) as pp_v,
            ):
                # ones columns for the softmax denominator, one memset per
                # parity slab
                nc.vector.memset(v2_sb[:, :, 0, HID:HID + 1], 1.0)
                nc.vector.memset(v2_sb[:, :, 1, HID:HID + 1], 1.0)

                # kT / qq: DoubleRow matmuls, drains paired into 1024-col copies
                drain_flip = 0
                for dst, w_sb, src in ((kT_sb, wkp_sb, hTp_sb),
                                       (qq_sb, wqp_sb, qTp_sb)):
                    for col in range(0, ROWS, 1024):
                        ps = pp_kq.tile([64, 1024], f32, tag="kq")
                        for h in range(2):
                            nc.tensor.matmul(
                                ps[:, h * 512:(h + 1) * 512], w_sb[:, :, :],
                                src[:, :, col + h * 512:col + (h + 1) * 512],
                                start=True, stop=True, perf_mode=DR)
                        if drain_flip % 2 == 0:
                            nc.vector.tensor_copy(dst[:, col:col + 1024],
                                                  ps[:])
                        else:
                            nc.scalar.copy(dst[:, col:col + 1024], ps[:])
                        drain_flip += 1

                # V projection (fp8 inputs, accumulate over the two e-halves),
                # jt pairs share one PSUM tile so drains are single 512-col
                # copies straight into the paired v2 layout
                for s in range(SEGS):
                    for sp in range(NPAIR):
                        ps = pp_v.tile([128, 2 * HID], f32, tag="v")
                        for par in range(2):
                            col = s * LH + (2 * sp + par) * 128
                            for e in range(2):
                                nc.tensor.matmul(
                                    ps[:, par * HID:(par + 1) * HID],
                                    hTp_sb[:, e, col:col + 128],
                                    wvp_sb[:, e, :],
                                    start=(e == 0), stop=(e == 1))
                        dst = v2_sb[:, s * NPAIR + sp, :, 0:HID]
                        if sp % 2 == 0:
                            nc.vector.tensor_copy(dst, ps[:])
                        else:
                            nc.scalar.copy(dst, ps[:])

            # ---------------- phase 2: attention + epilogue ----------------
            with (
                tc.tile_pool(name="pt", bufs=18) as pt_pool,
                tc.tile_pool(name="qrow", bufs=6) as q_pool,
                tc.tile_pool(name="ep", bufs=10) as ep_pool,
                tc.tile_pool(name="ep8", bufs=12) as ep8_pool,
                tc.tile_pool(name="st8", bufs=10) as st8_pool,
                tc.tile_pool(name="outp", bufs=6) as o_pool,
                tc.tile_pool(name="ps_st", bufs=2,
                             space=bass.MemorySpace.PSUM) as ps_st,
                tc.tile_pool(name="ps_att", bufs=2,
                             space=bass.MemorySpace.PSUM) as ps_att,
                tc.tile_pool(name="ps_fc", bufs=1,
                             space=bass.MemorySpace.PSUM) as ps_fc,
                tc.tile_pool(name="ps_tp", bufs=1,
                             space=bass.MemorySpace.PSUM) as ps_tp,
            ):
                chunks = [(s, ic) for s in range(SEGS) for ic in range(NIC)]
                pts = {}     # ci -> list of NPAIR paired P^T tiles

                def emit_score_exp(ci, jt):
                    s, ic = chunks[ci]
                    icol = s * LQ + ic * ICW
                    st = ps_st.tile([128, ICW], f32, tag="st")
                    for h in range(2):
                        nc.tensor.matmul(
                            st[:, h * 512:(h + 1) * 512],
                            kT_sb[:, s * LH + jt * 128:
                                  s * LH + (jt + 1) * 128],
                            qq_sb[:, icol + h * 512:icol + (h + 1) * 512],
                            start=True, stop=True)
                    if jt % 2 == 0:
                        pt2 = pt_pool.tile([128, 2, ICW], f8, tag="pt")
                        pts[ci].append(pt2)
                    nc.scalar.activation(pts[ci][jt // 2][:, jt % 2, :],
                                         st[:], AF.Exp,
                                         scale=SCALE, bias=nb3_sb[:])

                # prologue: first chunk's scores+exp
                pts[0] = []
                for jt in range(NJT):
                    emit_score_exp(0, jt)

                for ci, (s, ic) in enumerate(chunks):
                    if ci + 1 < len(chunks):
                        pts[ci + 1] = []
                    crow = s * LQ + ic * ICW
                    qc = q_pool.tile([128, NIL, HID], bf16, tag="q")
                    nc.sync.dma_start(
                        qc[:], qres_a[crow:crow + ICW, :].rearrange(
                            "(t p) f -> p t f", p=128))
                    oc = o_pool.tile([128, NIL, HID], bf16, tag="ot")
                    mva0 = st8_pool.tile([128, 2 * NIL], f32, tag="mva0")
                    xs = []
                    for il in range(NIL):
                        # P@V: DoubleRow over jt pairs, P^T pair stationary
                        att = ps_att.tile([128, HID + 1], f32, tag="att")
                        for sp in range(NPAIR):
                            nc.tensor.matmul(
                                att[:],
                                pts[ci][sp][:, :, il * 128:(il + 1) * 128],
                                v2_sb[:, s * NPAIR + sp, :, 0:HID + 1],
                                start=(sp == 0), stop=(sp == NPAIR - 1),
                                perf_mode=DR)
                        # LN is row-scale invariant: x0 = den*q + att
                        # normalizes identically to q + att/den
                        x0 = ep8_pool.tile([128, HID], bf16, tag="x0")
                        nc.vector.scalar_tensor_tensor(
                            x0[:], qc[:, il, :], att[:, HID:HID + 1].opt(),
                            att[:, 0:HID],
                            op0=Alu.mult, op1=Alu.add)
                        mv6 = st8_pool.tile([128, 6], f32, tag="mv6")
                        nc.vector.bn_stats(mv6[:], x0[:])
                        nc.vector.bn_aggr(mva0[:, 2 * il:2 * il + 2],
                                          mv6[:])
                        xs.append(x0)
                        # interleave next chunk's scores+exp so the PE and
                        # ACT streams stay dense across the chunk boundary
                        if ci + 1 < len(chunks):
                            emit_score_exp(ci + 1, 2 * il)
                            emit_score_exp(ci + 1, 2 * il + 1)

                    ln8a = st8_pool.tile([128, NIL], f32, tag="ln8a")
                    nc.scalar.activation(
                        ln8a[:].rearrange("p (t o) -> p t o", o=1),
                        mva0[:].rearrange("p (t o) -> p t o", o=2)[:, :, 1:2],
                        AF.Ln, bias=eps_sb[:])
                    rstd8a = st8_pool.tile([128, NIL], f32, tag="r8a")
                    nc.scalar.activation(rstd8a[:], ln8a[:], AF.Exp,
                                         scale=-0.5)

                    mva1 = st8_pool.tile([128, 2 * NIL], f32, tag="mva1")
                    ys = []
                    for il in range(NIL):
                        x0 = xs[il]
                        z = ep8_pool.tile([128, HID], bf16, tag="z")
                        nc.vector.tensor_scalar(
                            z[:], x0[:], mva0[:, 2 * il:2 * il + 1].opt(),
                            rstd8a[:, il:il + 1].opt(),
                            op0=Alu.subtract, op1=Alu.mult)
                        if apply0:
                            z2 = ep_pool.tile([128, HID], bf16, tag="z2")
                            nc.gpsimd.tensor_tensor(z2[:], z[:], n0w_sb[:],
                                                    op=Alu.mult)
                            z3 = ep_pool.tile([128, HID], bf16, tag="z3")
                            nc.gpsimd.tensor_tensor(z3[:], z2[:], n0b_sb[:],
                                                    op=Alu.add)
                            zf = z3
                        else:
                            zf = z
                        hres = ps_fc.tile([128, HID], f32, tag="fc")
                        nc.tensor.matmul(hres[:], one_sb[:], fb_sb[:],
                                         start=True, stop=False)
                        tp = ps_tp.tile([128, HID], bf16, tag="tp")
                        for hh in range(2):
                            nc.tensor.transpose(
                                tp[:, hh * 128:(hh + 1) * 128],
                                zf[:, hh * 128:(hh + 1) * 128],
                                idt_sb[:])
                        zT = ep_pool.tile([128, HID], bf16, tag="zT")
                        nc.vector.tensor_copy(zT[:], tp[:])
                        for hh in range(2):
                            nc.tensor.matmul(
                                hres[:], zT[:, hh * 128:(hh + 1) * 128],
                                fw_sb[:, hh * HID:(hh + 1) * HID],
                                start=False, stop=(hh == 1))
                        y0 = ep8_pool.tile([128, HID], bf16, tag="y0")
                        nc.vector.scalar_tensor_tensor(
                            y0[:], hres[:], 0.0, zf[:],
                            op0=Alu.max, op1=Alu.add)
                        mv6b = st8_pool.tile([128, 6], f32, tag="mv6b")
                        nc.vector.bn_stats(mv6b[:], y0[:])
                        nc.vector.bn_aggr(mva1[:, 2 * il:2 * il + 2],
                                          mv6b[:])
                        ys.append(y0)

                    ln8b = st8_pool.tile([128, NIL], f32, tag="ln8b")
                    nc.scalar.activation(
                        ln8b[:].rearrange("p (t o) -> p t o", o=1),
                        mva1[:].rearrange("p (t o) -> p t o", o=2)[:, :, 1:2],
                        AF.Ln, bias=eps_sb[:])
                    rstd8b = st8_pool.tile([128, NIL], f32, tag="r8b")
                    nc.scalar.activation(rstd8b[:], ln8b[:], AF.Exp,
                                         scale=-0.5)

                    mr8 = st8_pool.tile([128, NIL], f32, tag="b1")
                    nc.vector.tensor_tensor(
                        mr8[:],
                        mva1[:].rearrange("p (t o) -> p t o", o=2)[:, :, 0],
                        rstd8b[:], op=Alu.mult)
                    for il in range(NIL):
                        nc.vector.tensor_scalar(
                            oc[:, il, :], ys[il][:],
                            rstd8b[:, il:il + 1].opt(),
                            mr8[:, il:il + 1].opt(),
                            op0=Alu.mult, op1=Alu.subtract)
                    nc.sync.dma_start(
                        out_a[crow:crow + ICW, :].rearrange(
                            "(t p) f -> p t f", p=128), oc[:])
                    del pts[ci]

    nc.compile()
    return nc


def _get_nc(apply0: bool):
    key = (bool(apply0),)
    if key not in _built:
        _built[key] = _build(apply0)
    return _built[key]


def _pair(a):
    """[256, X] -> [128, 2, X] with contraction index ko*128 + p."""
    return np.ascontiguousarray(a.reshape(2, 128, -1).transpose(1, 0, 2))


def _shard(inputs, apply0):
    from concourse import mybir
    bf = mybir.dt.np(mybir.dt.bfloat16)
    f8 = mybir.dt.np(mybir.dt.float8e4)

    q = np.ascontiguousarray(np.asarray(inputs["q"], dtype=np.float32))
    h = np.ascontiguousarray(np.asarray(inputs["h"], dtype=np.float32))
    WQ = np.asarray(inputs["WQ"], dtype=np.float32)
    WK = np.asarray(inputs["WK"], dtype=np.float32)
    WV = np.asarray(inputs["WV"], dtype=np.float32)
    fcw = np.asarray(inputs["fc_w"], dtype=np.float32)
    fcb = np.asarray(inputs["fc_b"], dtype=np.float32)

    WQP = _pair(np.ascontiguousarray(WQ.T)).astype(f8)
    WKP = _pair(np.ascontiguousarray(WK.T)).astype(f8)
    WVP = _pair(np.ascontiguousarray(WV.T)).astype(f8)
    FCWT = np.ascontiguousarray(fcw.T).astype(bf)
    FCB = np.ascontiguousarray(fcb.reshape(1, HID)).astype(bf)
    IDT = np.eye(128, dtype=np.float32).astype(bf)

    in_maps = []
    for c in range(NCORES):
        sl = slice(c * ROWS, (c + 1) * ROWS)
        qT = np.ascontiguousarray(q[sl].T)   # [256, ROWS]
        hT = np.ascontiguousarray(h[sl].T)
        m = {
            "qTp": _pair(qT).astype(f8),
            "hTp": _pair(hT).astype(f8),
            "qres": q[sl].astype(bf),
            "WQP": WQP, "WKP": WKP, "WVP": WVP,
            "FCWT": FCWT, "FCB": FCB, "IDT": IDT,
        }
        if apply0:
            m["N0W"] = np.ascontiguousarray(
                np.broadcast_to(np.asarray(inputs["norm0_w"], np.float32),
                                (128, HID))).astype(bf)
            m["N0B"] = np.ascontiguousarray(
                np.broadcast_to(np.asarray(inputs["norm0_b"], np.float32),
                                (128, HID))).astype(bf)
        in_maps.append(m)
    return in_maps


def _run(inputs, trace=False, tmpdir=None):
    from concourse import bass_utils

    n0w = np.asarray(inputs["norm0_w"], np.float32)
    n0b = np.asarray(inputs["norm0_b"], np.float32)
    n1w = np.asarray(inputs["norm1_w"], np.float32)
    n1b = np.asarray(inputs["norm1_b"], np.float32)
    apply0 = not (np.allclose(n0w, 1.0) and np.allclose(n0b, 0.0))
    apply1 = not (np.allclose(n1w, 1.0) and np.allclose(n1b, 0.0))

    nc = _get_nc(apply0)
    in_maps = _shard(inputs, apply0)
    res = bass_utils.run_bass_kernel_spmd(
        nc, in_maps, core_ids=list(range(NCORES)), trace=trace,
        tmpdir=tmpdir)
    out = np.concatenate([np.asarray(res.results[c]["out"])
                          for c in range(NCORES)], axis=0).astype(np.float32)
    if apply1:
        out = out * n1w[None, :] + n1b[None, :]
    return out.astype(np.float32), res


def kernel(**inputs):
    out, _ = _run(inputs, trace=False)
    return out
